# revision 15
# baseline (speedup 1.0000x reference)
"""MinLSTM Trainium2 kernel (v2).

Problem: B=8, S=4096, In=512, H=512 (fp32).
    f_t = sigmoid(x @ W_f^T + b_f); i_t = sigmoid(x @ W_i^T + b_i)
    h_tilde = x @ W_h^T + b_h
    f_n = f_t / (f_t + i_t + eps); i_n = i_t / (f_t + i_t + eps)
    h_t = f_n * h_{t-1} + i_n * h_tilde   (scan over S)

Strategy: data-parallel over batch — 1 sample per NeuronCore (8 cores).
Per-core layout is transposed: [H on partitions, S on free dim].

v2 key changes vs the v1 baseline (which was DVE-bound on the ~6
cycle/elem iterative `reciprocal`):
  - custom fused DVE op MINLSTM_GATE_FN: fn = f * approx(1/(f+i)) via the
    BITWISE_NOT exponent-flip seed + one Chebyshev-Newton step — one DVE
    pass at 1 elem/cycle/lane instead of add+reciprocal+mult (~7.5x less
    DVE time for the normalization).
  - W_h and b_h negated on host so ACT emits -(x@W_h^T + b_h) for free;
    g = i_n*h_tilde is then one scalar_tensor_tensor: (fn - 1) * (-h2).
  - 2048-wide PSUM tiles (4 banks) so each ACT drain amortizes its fixed
    overhead over 2048 elements; sigmoid-set functions only (no ACT
    table switches).
  - bf16 everywhere downstream of the sigmoids; bf16 output (halves the
    store DMA).
"""

import numpy as np
import ml_dtypes

import concourse.bass as bass
import concourse.bacc as bacc
import concourse.tile as tile
from concourse import mybir
from concourse.bass import ts, ds
from concourse.bass_utils import run_bass_kernel_spmd

BF16 = ml_dtypes.bfloat16

B, S, IN, H = 8, 4096, 512, 512
KI = IN // 128        # 4 k-tiles of the contraction dim
KI2 = IN // 256       # 2 DoubleRow k-tiles (fp8 path)
HB = H // 128         # 4 h blocks (partition blocks)
CH = 2048             # S-chunk per PSUM tile (4 banks)
NCH = S // CH         # 2 chunks
MM = 512              # matmul free dim (1 PSUM bank)

USE_FP8 = True        # fp8e4m3 DoubleRow for the f/i gate matmuls
FP8_SC = 64.0         # weight pre-scale (undone via ACT scale=1/FP8_SC)
FP8 = ml_dtypes.float8_e4m3

# Chebyshev-minimax constants for the bitwise-not reciprocal seed
# (same interval as RECIP_APPROX_FAST_CONSTS in concourse.dve_ops).
_RC0 = -0.23549792
_RC1 = 2.0017324

_CACHE = {}


def _register_gate_fn():
    """Register the fused gate op: out = in0 * approx(1/(in0+in1)).

    s = f+i; seed y0 = bitcast(~s)*C0; y1 = y0*(C1 - s*y0); out = f*y1.
    7 ALU stages, one DVE instruction at 1 elem/cycle/lane (fp32 streams).
    Max rel err of fn vs exact f/(f+i): ~1.7e-3.
    """
    import concourse.dve_ops as D

    for op in D.OPS:
        if op.name == "MINLSTM_GATE_FN":
            return op

    from concourse.dve_spec import Spec, Src0, Src1, C0, C1, Bin, AluOp, lower
    from concourse.dve_uop import DveOpSpec

    _s = Src0 + Src1
    _ns = Bin(AluOp.BITWISE_NOT, _s, _s)
    _y0 = _ns * C0
    _y1 = _y0 * (C1 - _s * _y0)

    def _ref(in0, in1, s0, s1, imm2):
        s = (in0 + in1).astype(np.float32)
        not_s = (~s.view(np.int32)).view(np.float32)
        y0 = not_s * np.float32(s0)
        y1 = y0 * (np.float32(s1) - s * y0)
        return in0 * y1

    spec = Spec(body=Src0 * _y1, reference=_ref)
    shas = {}
    op = D.DveOp("MINLSTM_GATE_FN", spec, subdim=False, uops_sha=shas)
    D.OPS.append(op)
    D.CUSTOM_DVE_SPECS[op.name] = spec
    D._SUB_OPCODE_FOR_NAME[op.name] = D._CUSTOM_DVE_ROW_BASE + len(D.OPS) - 1
    opcode = D.get_dve_sub_opcode(op.name)
    for ver in ("v3", "v4"):
        s = DveOpSpec(
            name=op.name, opcode=opcode, uops=lower(spec, ver=ver), rd1_en=True
        )
        shas[ver] = s.sha(ver)
    return op


def build_minlstm_bass(repeat=1, use_fp8=None):
    if use_fp8 is None:
        use_fp8 = USE_FP8
    gate_fn_op = _register_gate_fn()

    nc = bacc.Bacc("TRN2", debug=False, num_devices=B)
    f32 = mybir.dt.float32
    bf16 = mybir.dt.bfloat16
    fp8 = mybir.dt.float8e4

    xT = nc.dram_tensor("xt", [KI, 128, S], bf16, kind="ExternalInput").ap()
    if use_fp8:
        xT8 = nc.dram_tensor(
            "xt8", [KI2, 128, 2, S], fp8, kind="ExternalInput").ap()
        wf8T = nc.dram_tensor(
            "wf8t", [KI2, 128, 2, H], fp8, kind="ExternalInput").ap()
        wi8T = nc.dram_tensor(
            "wi8t", [KI2, 128, 2, H], fp8, kind="ExternalInput").ap()
    else:
        wfT = nc.dram_tensor(
            "wft", [KI, 128, H], bf16, kind="ExternalInput").ap()
        wiT = nc.dram_tensor(
            "wit", [KI, 128, H], bf16, kind="ExternalInput").ap()
    whnT = nc.dram_tensor("whnt", [KI, 128, H], bf16, kind="ExternalInput").ap()
    bfb = nc.dram_tensor("bfb", [128, HB], f32, kind="ExternalInput").ap()
    bib = nc.dram_tensor("bib", [128, HB], f32, kind="ExternalInput").ap()
    bhnb = nc.dram_tensor("bhnb", [128, HB], f32, kind="ExternalInput").ap()
    h0b = nc.dram_tensor("h0b", [128, HB], f32, kind="ExternalInput").ap()
    outT = nc.dram_tensor("outt", [HB, 128, S], f32, kind="ExternalOutput").ap()

    Sig = mybir.ActivationFunctionType.Sigmoid
    Ident = mybir.ActivationFunctionType.Identity
    Alu = mybir.AluOpType

    with tile.TileContext(nc) as tc, nc.allow_low_precision(reason="bf16 gates"):
        with (
            tc.tile_pool(name="const", bufs=1) as const,
            tc.tile_pool(name="ps", bufs=2, space="PSUM") as ps,
            tc.tile_pool(name="gate32", bufs=2) as gate32,
            tc.tile_pool(name="gate16", bufs=2) as gate16,
            tc.tile_pool(name="hout", bufs=2) as hout,
        ):
            whn_sb = const.tile([128, KI, H], bf16, tag="whn")
            x_sb = const.tile([128, KI, S], bf16, tag="x")
            if use_fp8:
                wf8_sb = const.tile([128, KI2, 2, H], fp8, tag="wf8")
                wi8_sb = const.tile([128, KI2, 2, H], fp8, tag="wi8")
                x8_sb = const.tile([128, KI2, 2, S], fp8, tag="x8")
                for ki2 in range(KI2):
                    nc.sync.dma_start(
                        out=wf8_sb[:, ki2, :, :], in_=wf8T[ki2, :, :, :])
                    nc.sync.dma_start(
                        out=wi8_sb[:, ki2, :, :], in_=wi8T[ki2, :, :, :])
            else:
                wf_sb = const.tile([128, KI, H], bf16, tag="wf")
                wi_sb = const.tile([128, KI, H], bf16, tag="wi")
                for ki in range(KI):
                    nc.sync.dma_start(out=wf_sb[:, ki, :], in_=wfT[ki, :, :])
                    nc.sync.dma_start(out=wi_sb[:, ki, :], in_=wiT[ki, :, :])
            for ki in range(KI):
                nc.sync.dma_start(out=whn_sb[:, ki, :], in_=whnT[ki, :, :])
            for ch in range(NCH):
                if use_fp8:
                    for ki2 in range(KI2):
                        nc.sync.dma_start(
                            out=x8_sb[:, ki2, :, ts(ch, CH)],
                            in_=xT8[ki2, :, :, ts(ch, CH)])
                for ki in range(KI):
                    nc.sync.dma_start(
                        out=x_sb[:, ki, ts(ch, CH)], in_=xT[ki, :, ts(ch, CH)])
            bf_sb = const.tile([128, HB], f32, tag="bf")
            bi_sb = const.tile([128, HB], f32, tag="bi")
            bhn_sb = const.tile([128, HB], f32, tag="bhn")
            h0_sb = const.tile([128, HB], f32, tag="h0")
            nc.sync.dma_start(out=bf_sb, in_=bfb[:, :])
            nc.sync.dma_start(out=bi_sb, in_=bib[:, :])
            nc.sync.dma_start(out=bhn_sb, in_=bhnb[:, :])
            nc.sync.dma_start(out=h0_sb, in_=h0b[:, :])

            def gate_matmul(w_sb, hb, ch):
                pp = ps.tile([128, CH], f32, tag="pp")
                for ki in range(KI):
                    st, sp = (ki == 0), (ki == KI - 1)
                    for c in range(CH // MM):
                        nc.tensor.matmul(
                            pp[:, ts(c, MM)],
                            w_sb[:, ki, ds(hb * 128, 128)],
                            x_sb[:, ki, ds(ch * CH + c * MM, MM)],
                            start=st, stop=sp)
                return pp

            def gate_matmul_fp8(w8_sb, hb, ch):
                pp = ps.tile([128, CH], f32, tag="pp")
                for ki2 in range(KI2):
                    st, sp = (ki2 == 0), (ki2 == KI2 - 1)
                    for c in range(CH // MM):
                        nc.tensor.matmul(
                            pp[:, ts(c, MM)],
                            w8_sb[:, ki2, :, ds(hb * 128, 128)],
                            x8_sb[:, ki2, :, ds(ch * CH + c * MM, MM)],
                            start=st, stop=sp,
                            perf_mode=mybir.MatmulPerfMode.DoubleRow)
                return pp

            def body(_i=None):
                for hb in range(HB):
                    hh = hout.tile([128, S], f32, tag="hh")
                    for ch in range(NCH):
                        sc = 1.0 / FP8_SC if use_fp8 else 1.0
                        pph = gate_matmul(whn_sb, hb, ch)
                        htn = gate16.tile([128, CH], bf16, tag="htn")
                        nc.scalar.activation(
                            htn, pph, Ident, bias=bhn_sb[:, hb : hb + 1])
                        if use_fp8:
                            ppf = gate_matmul_fp8(wf8_sb, hb, ch)
                        else:
                            ppf = gate_matmul(wf_sb, hb, ch)
                        sf = gate32.tile([128, CH], f32, tag="sf")
                        nc.scalar.activation(
                            sf, ppf, Sig, bias=bf_sb[:, hb : hb + 1], scale=sc)
                        if use_fp8:
                            ppi = gate_matmul_fp8(wi8_sb, hb, ch)
                        else:
                            ppi = gate_matmul(wi_sb, hb, ch)
                        si = gate32.tile([128, CH], f32, tag="si")
                        nc.scalar.activation(
                            si, ppi, Sig, bias=bi_sb[:, hb : hb + 1], scale=sc)

                        fn = gate16.tile([128, CH], bf16, tag="fn")
                        nc.vector._custom_dve(
                            gate_fn_op, out=fn, in0=sf, in1=si,
                            s0=_RC0, s1=_RC1)
                        # (fn - 1) on the (otherwise idle) GPSIMD engine;
                        # g = (fn - 1) * (-h2) = (1 - fn) * h2 on DVE (bf16 2x).
                        inn = gate16.tile([128, CH], bf16, tag="inn")
                        nc.gpsimd.tensor_scalar(
                            inn, fn, 1.0, -1.0, Alu.mult, Alu.add)
                        g = gate16.tile([128, CH], bf16, tag="g")
                        nc.vector.tensor_tensor(g, inn, htn, Alu.mult)
                        init = (h0_sb[:, hb : hb + 1] if ch == 0
                                else hh[:, ch * CH - 1 : ch * CH])
                        nc.vector.tensor_tensor_scan(
                            hh[:, ts(ch, CH)], fn, g, init, Alu.mult, Alu.add)
                        nc.sync.dma_start(
                            out=outT[hb, :, ts(ch, CH)], in_=hh[:, ts(ch, CH)])

            if repeat == 1:
                body()
            else:
                with tc.For_i(0, repeat, 1) as _i:
                    body(_i)
    nc.compile()
    return nc


def _dr8(W):
    """[H, In] -> DoubleRow fp8 layout [KI2, 128, 2, H]: plane pl of ki2
    holds k-tile (2*ki2+pl)."""
    return np.ascontiguousarray(
        W.T.reshape(KI2, 2, 128, H).transpose(0, 2, 1, 3).astype(FP8))


def _prep_core_inputs(x, h_0, W_f, b_f, W_i, b_i, W_h, b_h):
    """Build per-core input maps (host-side shard + layout transform)."""
    use_fp8 = USE_FP8
    shared = {}
    if use_fp8:
        shared["wf8t"] = _dr8(W_f * FP8_SC)
        shared["wi8t"] = _dr8(W_i * FP8_SC)
    else:
        shared["wft"] = np.ascontiguousarray(
            W_f.T.reshape(KI, 128, H).astype(BF16))
        shared["wit"] = np.ascontiguousarray(
            W_i.T.reshape(KI, 128, H).astype(BF16))
    shared["whnt"] = np.ascontiguousarray(
        (-W_h).T.reshape(KI, 128, H).astype(BF16))
    shared["bfb"] = np.ascontiguousarray(
        b_f.reshape(HB, 128).T.astype(np.float32))
    shared["bib"] = np.ascontiguousarray(
        b_i.reshape(HB, 128).T.astype(np.float32))
    shared["bhnb"] = np.ascontiguousarray(
        (-b_h).reshape(HB, 128).T.astype(np.float32))
    in_maps = []
    for b in range(B):
        m = dict(shared)
        m["xt"] = np.ascontiguousarray(x[b].T.reshape(KI, 128, S).astype(BF16))
        if use_fp8:
            m["xt8"] = np.ascontiguousarray(
                x[b].T.reshape(KI2, 2, 128, S).transpose(0, 2, 1, 3)
                .astype(FP8))
        m["h0b"] = np.ascontiguousarray(
            h_0[b].reshape(HB, 128).T.astype(np.float32))
        in_maps.append(m)
    return in_maps


def _run(in_maps, trace=False, repeat=1):
    key = f"nc{repeat}_fp8{USE_FP8}"
    if key not in _CACHE:
        _CACHE[key] = build_minlstm_bass(repeat=repeat)
    return run_bass_kernel_spmd(
        _CACHE[key], in_maps, core_ids=list(range(B)), trace=trace)


def kernel(x, h_0, W_f, b_f, W_i, b_i, W_h, b_h):
    x = np.asarray(x, dtype=np.float32)
    h_0 = np.asarray(h_0, dtype=np.float32)
    in_maps = _prep_core_inputs(
        x, h_0,
        np.asarray(W_f, np.float32), np.asarray(b_f, np.float32),
        np.asarray(W_i, np.float32), np.asarray(b_i, np.float32),
        np.asarray(W_h, np.float32), np.asarray(b_h, np.float32))
    res = _run(in_maps)
    out = np.empty((B, S, H), dtype=np.float32)
    for b in range(B):
        outt = res.results[b]["outt"]  # [HB, 128, S] bf16
        out[b] = outt.reshape(H, S).T.astype(np.float32)
    return out


# revision 19
# speedup vs baseline: 1.2459x; 1.2459x over previous
"""MinLSTM Trainium2 kernel (v2).

Problem: B=8, S=4096, In=512, H=512 (fp32).
    f_t = sigmoid(x @ W_f^T + b_f); i_t = sigmoid(x @ W_i^T + b_i)
    h_tilde = x @ W_h^T + b_h
    f_n = f_t / (f_t + i_t + eps); i_n = i_t / (f_t + i_t + eps)
    h_t = f_n * h_{t-1} + i_n * h_tilde   (scan over S)

Strategy: data-parallel over batch — 1 sample per NeuronCore (8 cores).
Per-core layout is transposed: [H on partitions, S on free dim].

v2 key changes vs the v1 baseline (which was DVE-bound on the ~6
cycle/elem iterative `reciprocal`):
  - custom fused DVE op MINLSTM_GATE_FN: fn = f * approx(1/(f+i)) via the
    BITWISE_NOT exponent-flip seed + one Chebyshev-Newton step — one DVE
    pass at 1 elem/cycle/lane instead of add+reciprocal+mult (~7.5x less
    DVE time for the normalization).
  - W_h and b_h negated on host so ACT emits -(x@W_h^T + b_h) for free;
    g = i_n*h_tilde is then one scalar_tensor_tensor: (fn - 1) * (-h2).
  - 2048-wide PSUM tiles (4 banks) so each ACT drain amortizes its fixed
    overhead over 2048 elements; sigmoid-set functions only (no ACT
    table switches).
  - bf16 everywhere downstream of the sigmoids; bf16 output (halves the
    store DMA).
"""

import numpy as np
import ml_dtypes

import concourse.bass as bass
import concourse.bacc as bacc
import concourse.tile as tile
from concourse import mybir
from concourse.bass import ts, ds
from concourse.bass_utils import run_bass_kernel_spmd

BF16 = ml_dtypes.bfloat16

B, S, IN, H = 8, 4096, 512, 512
KI = IN // 128        # 4 k-tiles of the contraction dim
KI2 = IN // 256       # 2 DoubleRow k-tiles (fp8 path)
HB = H // 128         # 4 h blocks (partition blocks)
CH = 2048             # S-chunk per PSUM tile (4 banks)
NCH = S // CH         # 2 chunks
MM = 512              # matmul free dim (1 PSUM bank)

USE_FP8 = True        # fp8e4m3 DoubleRow for the f/i gate matmuls
FP8_SC = 64.0         # weight pre-scale (undone via ACT scale=1/FP8_SC)
FP8 = ml_dtypes.float8_e4m3

# Chebyshev-minimax constants for the bitwise-not reciprocal seed
# (same interval as RECIP_APPROX_FAST_CONSTS in concourse.dve_ops).
_RC0 = -0.23549792
_RC1 = 2.0017324

_CACHE = {}


def _register_gate_fn():
    """Register the fused gate op: out = in0 * approx(1/(in0+in1)).

    s = f+i; seed y0 = bitcast(~s)*C0; y1 = y0*(C1 - s*y0); out = f*y1.
    7 ALU stages, one DVE instruction at 1 elem/cycle/lane (fp32 streams).
    Max rel err of fn vs exact f/(f+i): ~1.7e-3.
    """
    import concourse.dve_ops as D

    for op in D.OPS:
        if op.name == "MINLSTM_GATE_FN":
            return op

    from concourse.dve_spec import Spec, Src0, Src1, C0, C1, Bin, AluOp, lower
    from concourse.dve_uop import DveOpSpec

    _s = Src0 + Src1
    _ns = Bin(AluOp.BITWISE_NOT, _s, _s)
    _y0 = _ns * C0
    _y1 = _y0 * (C1 - _s * _y0)

    def _ref(in0, in1, s0, s1, imm2):
        s = (in0 + in1).astype(np.float32)
        not_s = (~s.view(np.int32)).view(np.float32)
        y0 = not_s * np.float32(s0)
        y1 = y0 * (np.float32(s1) - s * y0)
        return in0 * y1

    spec = Spec(body=Src0 * _y1, reference=_ref)
    shas = {}
    op = D.DveOp("MINLSTM_GATE_FN", spec, subdim=False, uops_sha=shas)
    D.OPS.append(op)
    D.CUSTOM_DVE_SPECS[op.name] = spec
    D._SUB_OPCODE_FOR_NAME[op.name] = D._CUSTOM_DVE_ROW_BASE + len(D.OPS) - 1
    opcode = D.get_dve_sub_opcode(op.name)
    for ver in ("v3", "v4"):
        s = DveOpSpec(
            name=op.name, opcode=opcode, uops=lower(spec, ver=ver), rd1_en=True
        )
        shas[ver] = s.sha(ver)
    return op


def build_minlstm_bass(repeat=1, use_fp8=None):
    if use_fp8 is None:
        use_fp8 = USE_FP8
    gate_fn_op = _register_gate_fn()

    nc = bacc.Bacc("TRN2", debug=False, num_devices=B)
    f32 = mybir.dt.float32
    bf16 = mybir.dt.bfloat16
    fp8 = mybir.dt.float8e4

    xT = nc.dram_tensor("xt", [KI, 128, S], bf16, kind="ExternalInput").ap()
    if use_fp8:
        xT8 = nc.dram_tensor(
            "xt8", [KI2, 128, 2, S], fp8, kind="ExternalInput").ap()
        wf8T = nc.dram_tensor(
            "wf8t", [KI2, 128, 2, H], fp8, kind="ExternalInput").ap()
        wi8T = nc.dram_tensor(
            "wi8t", [KI2, 128, 2, H], fp8, kind="ExternalInput").ap()
    else:
        wfT = nc.dram_tensor(
            "wft", [KI, 128, H], bf16, kind="ExternalInput").ap()
        wiT = nc.dram_tensor(
            "wit", [KI, 128, H], bf16, kind="ExternalInput").ap()
    whnT = nc.dram_tensor("whnt", [KI, 128, H], bf16, kind="ExternalInput").ap()
    bfb = nc.dram_tensor("bfb", [128, HB], f32, kind="ExternalInput").ap()
    bib = nc.dram_tensor("bib", [128, HB], f32, kind="ExternalInput").ap()
    bhnb = nc.dram_tensor("bhnb", [128, HB], f32, kind="ExternalInput").ap()
    h0b = nc.dram_tensor("h0b", [128, HB], f32, kind="ExternalInput").ap()
    outT = nc.dram_tensor("outt", [HB, 128, S], f32, kind="ExternalOutput").ap()

    Sig = mybir.ActivationFunctionType.Sigmoid
    Ident = mybir.ActivationFunctionType.Identity
    Alu = mybir.AluOpType

    with tile.TileContext(nc) as tc, nc.allow_low_precision(reason="bf16 gates"):
        with (
            tc.tile_pool(name="const", bufs=1) as const,
            tc.tile_pool(name="ps", bufs=2, space="PSUM") as ps,
            tc.tile_pool(name="gate32", bufs=2) as gate32,
            tc.tile_pool(name="gate16", bufs=3) as gate16,
            tc.tile_pool(name="hout", bufs=2) as hout,
        ):
            whn_sb = const.tile([128, KI, H], bf16, tag="whn")
            x_sb = const.tile([128, KI, S], bf16, tag="x")
            if use_fp8:
                wf8_sb = const.tile([128, KI2, 2, H], fp8, tag="wf8")
                wi8_sb = const.tile([128, KI2, 2, H], fp8, tag="wi8")
                x8_sb = const.tile([128, KI2, 2, S], fp8, tag="x8")
                for ki2 in range(KI2):
                    nc.sync.dma_start(
                        out=wf8_sb[:, ki2, :, :], in_=wf8T[ki2, :, :, :])
                    nc.sync.dma_start(
                        out=wi8_sb[:, ki2, :, :], in_=wi8T[ki2, :, :, :])
            else:
                wf_sb = const.tile([128, KI, H], bf16, tag="wf")
                wi_sb = const.tile([128, KI, H], bf16, tag="wi")
                for ki in range(KI):
                    nc.sync.dma_start(out=wf_sb[:, ki, :], in_=wfT[ki, :, :])
                    nc.sync.dma_start(out=wi_sb[:, ki, :], in_=wiT[ki, :, :])
            for ki in range(KI):
                nc.sync.dma_start(out=whn_sb[:, ki, :], in_=whnT[ki, :, :])
            for ch in range(NCH):
                if use_fp8:
                    for ki2 in range(KI2):
                        nc.sync.dma_start(
                            out=x8_sb[:, ki2, :, ts(ch, CH)],
                            in_=xT8[ki2, :, :, ts(ch, CH)])
                for ki in range(KI):
                    nc.sync.dma_start(
                        out=x_sb[:, ki, ts(ch, CH)], in_=xT[ki, :, ts(ch, CH)])
            bf_sb = const.tile([128, HB], f32, tag="bf")
            bi_sb = const.tile([128, HB], f32, tag="bi")
            bhn_sb = const.tile([128, HB], f32, tag="bhn")
            h0_sb = const.tile([128, HB], f32, tag="h0")
            nc.sync.dma_start(out=bf_sb, in_=bfb[:, :])
            nc.sync.dma_start(out=bi_sb, in_=bib[:, :])
            nc.sync.dma_start(out=bhn_sb, in_=bhnb[:, :])
            nc.sync.dma_start(out=h0_sb, in_=h0b[:, :])

            def gate_matmul(w_sb, hb, ch):
                pp = ps.tile([128, CH], f32, tag="pp")
                for ki in range(KI):
                    st, sp = (ki == 0), (ki == KI - 1)
                    for c in range(CH // MM):
                        nc.tensor.matmul(
                            pp[:, ts(c, MM)],
                            w_sb[:, ki, ds(hb * 128, 128)],
                            x_sb[:, ki, ds(ch * CH + c * MM, MM)],
                            start=st, stop=sp)
                return pp

            def gate_matmul_fp8(w8_sb, hb, ch):
                pp = ps.tile([128, CH], f32, tag="pp")
                for ki2 in range(KI2):
                    st, sp = (ki2 == 0), (ki2 == KI2 - 1)
                    for c in range(CH // MM):
                        nc.tensor.matmul(
                            pp[:, ts(c, MM)],
                            w8_sb[:, ki2, :, ds(hb * 128, 128)],
                            x8_sb[:, ki2, :, ds(ch * CH + c * MM, MM)],
                            start=st, stop=sp,
                            perf_mode=mybir.MatmulPerfMode.DoubleRow)
                return pp

            def emit_scan(p, last):
                """Finish pending chunk p: g on DVE, scan, out-DMA.

                For non-last chunks g reads inn (computed on GPSIMD one
                chunk ago — the cross-engine hop is already hidden); for
                the final chunk use a single STT so the tail chain skips
                the GPSIMD hop.
                """
                hb, ch, hh, fn, htn, inn = p
                g = gate16.tile([128, CH], bf16, tag="g")
                if last:
                    nc.vector.scalar_tensor_tensor(
                        g, fn, 1.0, htn, Alu.subtract, Alu.mult)
                else:
                    nc.vector.tensor_tensor(g, inn, htn, Alu.mult)
                init = (h0_sb[:, hb : hb + 1] if ch == 0
                        else hh[:, ch * CH - 1 : ch * CH])
                nc.vector.tensor_tensor_scan(
                    hh[:, ts(ch, CH)], fn, g, init, Alu.mult, Alu.add)
                nc.sync.dma_start(
                    out=outT[hb, :, ts(ch, CH)], in_=hh[:, ts(ch, CH)])

            def body(_i=None):
                pending = None
                hh = None
                for hb in range(HB):
                    hh = hout.tile([128, S], f32, tag="hh")
                    for ch in range(NCH):
                        sc = 1.0 / FP8_SC if use_fp8 else 1.0
                        if use_fp8:
                            ppf = gate_matmul_fp8(wf8_sb, hb, ch)
                        else:
                            ppf = gate_matmul(wf_sb, hb, ch)
                        sf = gate32.tile([128, CH], f32, tag="sf")
                        nc.scalar.activation(
                            sf, ppf, Sig, bias=bf_sb[:, hb : hb + 1], scale=sc)
                        if use_fp8:
                            ppi = gate_matmul_fp8(wi8_sb, hb, ch)
                        else:
                            ppi = gate_matmul(wi_sb, hb, ch)
                        si = gate32.tile([128, CH], f32, tag="si")
                        nc.scalar.activation(
                            si, ppi, Sig, bias=bi_sb[:, hb : hb + 1], scale=sc)
                        pph = gate_matmul(whn_sb, hb, ch)
                        htn = gate16.tile([128, CH], bf16, tag="htn")
                        nc.scalar.activation(
                            htn, pph, Ident, bias=bhn_sb[:, hb : hb + 1])

                        fn = gate16.tile([128, CH], bf16, tag="fn")
                        nc.vector._custom_dve(
                            gate_fn_op, out=fn, in0=sf, in1=si,
                            s0=_RC0, s1=_RC1)
                        inn = gate16.tile([128, CH], bf16, tag="inn")
                        nc.gpsimd.tensor_scalar(
                            inn, fn, 1.0, -1.0, Alu.mult, Alu.add)
                        if pending is not None:
                            emit_scan(pending, last=False)
                        pending = (hb, ch, hh, fn, htn, inn)
                emit_scan(pending, last=True)

            if repeat == 1:
                body()
            else:
                with tc.For_i(0, repeat, 1) as _i:
                    body(_i)
    nc.compile()
    return nc


def _dr8(W):
    """[H, In] -> DoubleRow fp8 layout [KI2, 128, 2, H]: plane pl of ki2
    holds k-tile (2*ki2+pl)."""
    return np.ascontiguousarray(
        W.T.reshape(KI2, 2, 128, H).transpose(0, 2, 1, 3).astype(FP8))


def _prep_core_inputs(x, h_0, W_f, b_f, W_i, b_i, W_h, b_h):
    """Build per-core input maps (host-side shard + layout transform)."""
    use_fp8 = USE_FP8
    shared = {}
    if use_fp8:
        shared["wf8t"] = _dr8(W_f * FP8_SC)
        shared["wi8t"] = _dr8(W_i * FP8_SC)
    else:
        shared["wft"] = np.ascontiguousarray(
            W_f.T.reshape(KI, 128, H).astype(BF16))
        shared["wit"] = np.ascontiguousarray(
            W_i.T.reshape(KI, 128, H).astype(BF16))
    shared["whnt"] = np.ascontiguousarray(
        (-W_h).T.reshape(KI, 128, H).astype(BF16))
    shared["bfb"] = np.ascontiguousarray(
        b_f.reshape(HB, 128).T.astype(np.float32))
    shared["bib"] = np.ascontiguousarray(
        b_i.reshape(HB, 128).T.astype(np.float32))
    shared["bhnb"] = np.ascontiguousarray(
        (-b_h).reshape(HB, 128).T.astype(np.float32))
    in_maps = []
    for b in range(B):
        m = dict(shared)
        m["xt"] = np.ascontiguousarray(x[b].T.reshape(KI, 128, S).astype(BF16))
        if use_fp8:
            m["xt8"] = np.ascontiguousarray(
                x[b].T.reshape(KI2, 2, 128, S).transpose(0, 2, 1, 3)
                .astype(FP8))
        m["h0b"] = np.ascontiguousarray(
            h_0[b].reshape(HB, 128).T.astype(np.float32))
        in_maps.append(m)
    return in_maps


def _run(in_maps, trace=False, repeat=1):
    key = f"nc{repeat}_fp8{USE_FP8}"
    if key not in _CACHE:
        _CACHE[key] = build_minlstm_bass(repeat=repeat)
    return run_bass_kernel_spmd(
        _CACHE[key], in_maps, core_ids=list(range(B)), trace=trace)


def kernel(x, h_0, W_f, b_f, W_i, b_i, W_h, b_h):
    x = np.asarray(x, dtype=np.float32)
    h_0 = np.asarray(h_0, dtype=np.float32)
    in_maps = _prep_core_inputs(
        x, h_0,
        np.asarray(W_f, np.float32), np.asarray(b_f, np.float32),
        np.asarray(W_i, np.float32), np.asarray(b_i, np.float32),
        np.asarray(W_h, np.float32), np.asarray(b_h, np.float32))
    res = _run(in_maps)
    out = np.empty((B, S, H), dtype=np.float32)
    for b in range(B):
        outt = res.results[b]["outt"]  # [HB, 128, S] bf16
        out[b] = outt.reshape(H, S).T.astype(np.float32)
    return out
